# revision 10
# baseline (speedup 1.0000x reference)
"""Trainium2 Bass kernel for nn_NonsharedPatchEmbed_86827058856432.

Computes, for a patchified [64, 3, 224, 224] fp32 image batch,

    out[b, p, o] = sum_i patches[b, p, i] * W[p, o, i] + bias[p, o]

with 196 independent Linear(768->768) layers (one per patch).

The problem is HBM-bound on W traffic (196*768*768 elements, each used
once per core under patch sharding), so W and the activations are cast to
bf16 on the host: this halves the dominant DMA bytes and quarters the
tensor-engine stream time (bf16 moves 1 col/cycle vs fp32's 4). PSUM
accumulation stays fp32; measured end-to-end relative error is ~4e-3,
well inside the 2e-2 gate.

Distribution: 196 = 8 * 24.5, so each core gets 24 full patches plus ONE
HALF of a shared patch (384 of its 768 outputs): patches 0-191 go 24 per
core, and patches 192-195 are split into 8 output-halves, one per core.
Every core therefore reads exactly 24.5/196 of W -- perfect balance, and
the per-core DMA-engine pool (16 engines x ~22.4 GB/s ~= 360 GB/s) is the
roofline. The half-patch job runs LAST so the post-last-W drain (compute +
PSUM copy + output write) is ~4x smaller than a full pair's.

Per-core kernel (column-tiled pairs):
  - 12 pairs of full patches, then the half-patch job.
  - For each pair, patch A owns PSUM partitions 0-63 (tile_position (0, 0)),
    patch B owns partitions 64-127 ((0, 64)). Each streams its own W^T as
    the moving operand; the batch activations (aT chunks, [128 x 64]) are
    the stationary operand.
  - The bias is applied with a K=2 bf16 matmul (ones x [bias_hi; bias_lo])
    that *starts* each PSUM accumulation group; the hi/lo split keeps the
    bias contribution bit-accurate and absorbs the PSUM WAR dependency.
  - A single HWDGE ring caps near ~200 GB/s, well below the 16-engine DMA
    pool, so W is split across BOTH rings every patch: chunks 0-2 ride the
    SP ring, chunks 3-5 the ACT ring. Activations+bias ride SP, outputs
    ride ACT, keeping the two rings byte-balanced (~17 MB each).
  - W dma_starts are issued before act/bias in each iteration so the
    big descriptors hit the queues first.

Layouts per core:
  aT  [128, 25, 6, 64]  bf16  aT[i, p, c, b] = patches[b, patch(p), 128c+i]
                              (p = 0..23 full patches, p = 24 half patch)
  Wt  [24, 128, 6, 768] bf16  Wt[p, i, c, o] = W[patch(p), o, 128c+i]
  Wh  [128, 6, 384]     bf16  half-patch W slice (output half `hp_off`)
  bhl [2, 24, 768]      bf16  bias split as hi + lo
  bh  [2, 384]          bf16  half-patch bias hi + lo
  outp [12, 128, 768]   bf16  pair j rows 0-63 -> patch 2j, 64-127 -> 2j+1
  outh [64, 384]        bf16  half-patch outputs
"""

import numpy as np
import ml_dtypes

import concourse.tile as tile
import concourse.mybir as mybir
from concourse import bacc
from concourse.bass_utils import run_bass_kernel_spmd

f32 = mybir.dt.float32
bf16 = mybir.dt.bfloat16

N_CORES = 8
B = 64            # batch
D = 768           # in/out feature dim
HD = 384          # half of D (half-patch job width)
NP = 196          # real patches
FPC = 24          # full patches per core (8*24 = 192)
NPAIR = FPC // 2  # 12 pairs
NCHUNK = 6        # 768 / 128 contraction chunks

LAST_RESULTS = None    # BassKernelResults of the most recent run (for test.py)

_NC_CACHE = {}


def _build():
    nc = bacc.Bacc()
    aT = nc.declare_dram_parameter("aT", [128, FPC + 1, NCHUNK, B], bf16, isOutput=False)
    Wt = nc.declare_dram_parameter("Wt", [FPC, 128, NCHUNK, D], bf16, isOutput=False)
    Wh = nc.declare_dram_parameter("Wh", [128, NCHUNK, HD], bf16, isOutput=False)
    bhl = nc.declare_dram_parameter("bhl", [2, FPC, D], bf16, isOutput=False)
    bh = nc.declare_dram_parameter("bh", [2, HD], bf16, isOutput=False)
    outp = nc.declare_dram_parameter("outp", [NPAIR, 2 * B, D], bf16, isOutput=True)
    outh = nc.declare_dram_parameter("outh", [B, HD], bf16, isOutput=True)

    with tile.TileContext(nc) as tc:
        with (
            tc.tile_pool(name="const", bufs=1) as cpool,
            tc.tile_pool(name="a", bufs=6) as apool,
            tc.tile_pool(name="w", bufs=8) as wpool,
            tc.tile_pool(name="o", bufs=3) as opool,
            tc.tile_pool(name="ps", bufs=3, space="PSUM") as pspool,
            tc.tile_pool(name="psh", bufs=1, space="PSUM") as pshpool,
        ):
            ones = cpool.tile([2, B], bf16)
            nc.vector.memset(ones[:], 1.0)

            slices = [(0, 512), (512, 768)]

            def wtile(p):
                # Split each patch's W tile across BOTH HWDGE rings: a single
                # ring caps out near ~200 GB/s, well below the per-core DMA
                # engine pool, so W (the dominant traffic) must ride both.
                t = wpool.tile([128, NCHUNK, D], bf16, tag="wt")
                h = NCHUNK // 2
                nc.sync.dma_start(t[:, :h], Wt[p, :, :h])
                nc.scalar.dma_start(t[:, h:], Wt[p, :, h:])
                return t

            for j in range(NPAIR):
                p0, p1 = 2 * j, 2 * j + 1
                wt0 = wtile(p0)
                wt1 = wtile(p1)
                at = apool.tile([128, 2, NCHUNK, B], bf16, tag="at")
                tb = apool.tile([2, 2, D], bf16, tag="tb")
                nc.sync.dma_start(at[:], aT[:, p0:p0 + 2])
                nc.sync.dma_start(tb[:], bhl[:, p0:p0 + 2, :])
                a0 = at[:, 0]
                a1 = at[:, 1]

                pt = pspool.tile([2 * B, D], f32, tag="pt")
                for (o0, o1) in slices:
                    nc.tensor.matmul(
                        pt[:B, o0:o1], ones[:], tb[:, 0, o0:o1],
                        start=True, stop=False, tile_position=(0, 0),
                    )
                    nc.tensor.matmul(
                        pt[B:, o0:o1], ones[:], tb[:, 1, o0:o1],
                        start=True, stop=False, tile_position=(0, B),
                    )
                for c in range(NCHUNK):
                    for (o0, o1) in slices:
                        nc.tensor.matmul(
                            pt[:B, o0:o1], a0[:, c, :], wt0[:, c, o0:o1],
                            start=False, stop=(c == NCHUNK - 1),
                            tile_position=(0, 0),
                        )
                        nc.tensor.matmul(
                            pt[B:, o0:o1], a1[:, c, :], wt1[:, c, o0:o1],
                            start=False, stop=(c == NCHUNK - 1),
                            tile_position=(0, B),
                        )
                ob = opool.tile([2 * B, D], bf16, tag="ob")
                nc.vector.tensor_copy(ob[:], pt[:])
                nc.scalar.dma_start(outp[j], ob[:])

            # Half-patch job: one patch, HD of its D outputs, runs last so
            # the tail after the final W byte is short.
            wh = wpool.tile([128, NCHUNK, HD], bf16, tag="wh")
            hh = NCHUNK // 2
            nc.sync.dma_start(wh[:, :hh], Wh[:, :hh])
            nc.scalar.dma_start(wh[:, hh:], Wh[:, hh:])
            ah = apool.tile([128, 1, NCHUNK, B], bf16, tag="ah")
            tbh = apool.tile([2, HD], bf16, tag="tbh")
            nc.sync.dma_start(ah[:], aT[:, FPC:FPC + 1])
            nc.sync.dma_start(tbh[:], bh[:, :])

            ph = pshpool.tile([B, HD], f32, tag="ph")
            nc.tensor.matmul(
                ph[:, :], ones[:], tbh[:, :],
                start=True, stop=False, tile_position=(0, 0),
            )
            for c in range(NCHUNK):
                nc.tensor.matmul(
                    ph[:, :], ah[:, 0, c, :], wh[:, c, :],
                    start=False, stop=(c == NCHUNK - 1),
                    tile_position=(0, 0),
                )
            oh = opool.tile([B, HD], bf16, tag="oh")
            nc.vector.tensor_copy(oh[:], ph[:])
            nc.scalar.dma_start(outh[:, :], oh[:])

    nc.finalize()
    return nc


def _patchify(x):
    # [B, C, H, W] -> [B, 196, 768] in MAE ordering (n c h p w q -> n h w p q c)
    Bn, C, H, Wd = x.shape
    h = H // 16
    xr = x.reshape(Bn, C, h, 16, h, 16)
    xr = np.transpose(xr, (0, 2, 4, 3, 5, 1))
    return xr.reshape(Bn, h * h, 16 * 16 * C)


def _bias_hilo(v):
    hi = v.astype(ml_dtypes.bfloat16)
    lo = (v - hi.astype(np.float32)).astype(ml_dtypes.bfloat16)
    return np.ascontiguousarray(np.stack([hi, lo], axis=0))


def kernel(x, W, b, _trace=False, _tmpdir=None):
    global LAST_RESULTS

    x = np.asarray(x, dtype=np.float32)
    W = np.asarray(W, dtype=np.float32)
    b = np.asarray(b, dtype=np.float32)

    patches = _patchify(x).astype(ml_dtypes.bfloat16)   # [64, 196, 768]
    Wb = W.astype(ml_dtypes.bfloat16)                   # [196, 768, 768]

    in_maps = []
    for k in range(N_CORES):
        idx = np.arange(k * FPC, (k + 1) * FPC)         # full patches
        hp = 8 * FPC + k // 2                           # shared half patch
        ho = (k % 2) * HD                               # its output offset

        psl = patches[:, list(idx) + [hp], :]           # [64, 25, 768]
        aT = np.ascontiguousarray(
            psl.transpose(2, 1, 0)                      # [768, 25, 64]
            .reshape(NCHUNK, 128, FPC + 1, B)
            .transpose(1, 2, 0, 3)                      # [128, 25, 6, 64]
        )
        wsl = Wb[idx]                                   # [24, 768, 768]
        Wt = np.ascontiguousarray(
            wsl.transpose(0, 2, 1)                      # [24, 768(i), 768(o)]
            .reshape(FPC, NCHUNK, 128, D)
            .transpose(0, 2, 1, 3)                      # [24, 128, 6, 768]
        )
        Wh = np.ascontiguousarray(
            Wb[hp, ho:ho + HD, :]                       # [384(o), 768(i)]
            .transpose(1, 0)                            # [768(i), 384(o)]
            .reshape(NCHUNK, 128, HD)
            .transpose(1, 0, 2)                         # [128, 6, 384]
        )
        in_maps.append({
            "aT": aT, "Wt": Wt, "Wh": Wh,
            "bhl": _bias_hilo(b[idx]), "bh": _bias_hilo(b[hp, ho:ho + HD]),
        })

    if "F" not in _NC_CACHE:
        _NC_CACHE["F"] = _build()
    nc = _NC_CACHE["F"]

    res = run_bass_kernel_spmd(
        nc, in_maps, list(range(N_CORES)), trace=_trace, tmpdir=_tmpdir
    )
    LAST_RESULTS = res

    out = np.empty((B, N_CORES * FPC + 4, D), dtype=np.float32)
    for k in range(N_CORES):
        op = res.results[k]["outp"].astype(np.float32)  # [12, 128, 768]
        out[:, k * FPC:(k + 1) * FPC, :] = (
            op.reshape(FPC, B, D).transpose(1, 0, 2)
        )
        hp = 8 * FPC + k // 2
        ho = (k % 2) * HD
        out[:, hp, ho:ho + HD] = res.results[k]["outh"].astype(np.float32)
    return np.ascontiguousarray(out[:, :NP, :])


# revision 13
# speedup vs baseline: 1.0149x; 1.0149x over previous
"""Trainium2 Bass kernel for nn_NonsharedPatchEmbed_86827058856432.

Computes, for a patchified [64, 3, 224, 224] fp32 image batch,

    out[b, p, o] = sum_i patches[b, p, i] * W[p, o, i] + bias[p, o]

with 196 independent Linear(768->768) layers (one per patch).

The problem is HBM-bound on W traffic (196*768*768 elements, each used
once per core under patch sharding), so W and the activations are cast to
bf16 on the host: this halves the dominant DMA bytes and quarters the
tensor-engine stream time (bf16 moves 1 col/cycle vs fp32's 4). PSUM
accumulation stays fp32; measured end-to-end relative error is ~4e-3,
well inside the 2e-2 gate.

Distribution: 196 = 8 * 24.5, so each core gets 24 full patches plus ONE
HALF of a shared patch (384 of its 768 outputs): patches 0-191 go 24 per
core, and patches 192-195 are split into 8 output-halves, one per core.
Every core therefore reads exactly 24.5/196 of W -- perfect balance, and
the per-core DMA-engine pool (16 engines x ~22.4 GB/s ~= 360 GB/s) is the
roofline. The half-patch job runs LAST so the post-last-W drain (compute +
PSUM copy + output write) is ~4x smaller than a full pair's.

Per-core kernel (column-tiled pairs):
  - 12 pairs of full patches, then the half-patch job.
  - For each pair, patch A owns PSUM partitions 0-63 (tile_position (0, 0)),
    patch B owns partitions 64-127 ((0, 64)). Each streams its own W^T as
    the moving operand; the batch activations (aT chunks, [128 x 64]) are
    the stationary operand.
  - The bias is applied with a K=2 bf16 matmul (ones x [bias_hi; bias_lo])
    that *starts* each PSUM accumulation group; the hi/lo split keeps the
    bias contribution bit-accurate and absorbs the PSUM WAR dependency.
  - A single HWDGE ring caps near ~200 GB/s, well below the 16-engine DMA
    pool, so W is split across BOTH rings every patch: chunks 0-2 ride the
    SP ring, chunks 3-5 the ACT ring. Activations+bias ride SP, outputs
    ride ACT, keeping the two rings byte-balanced (~17 MB each).
  - W dma_starts are issued before act/bias in each iteration so the
    big descriptors hit the queues first.

Layouts per core:
  aT  [128, 25, 6, 64]  bf16  aT[i, p, c, b] = patches[b, patch(p), 128c+i]
                              (p = 0..23 full patches, p = 24 half patch)
  Wt  [24, 128, 6, 768] bf16  Wt[p, i, c, o] = W[patch(p), o, 128c+i]
  Wh  [128, 6, 384]     bf16  half-patch W slice (output half `hp_off`)
  bhl [2, 24, 768]      bf16  bias split as hi + lo
  bh  [2, 384]          bf16  half-patch bias hi + lo
  outp [12, 128, 768]   bf16  pair j rows 0-63 -> patch 2j, 64-127 -> 2j+1
  outh [64, 384]        bf16  half-patch outputs
"""

import numpy as np
import ml_dtypes

import concourse.tile as tile
import concourse.mybir as mybir
from concourse import bacc
from concourse.bass_utils import run_bass_kernel_spmd

f32 = mybir.dt.float32
bf16 = mybir.dt.bfloat16

N_CORES = 8
B = 64            # batch
D = 768           # in/out feature dim
HD = 384          # half of D (half-patch job width)
NP = 196          # real patches
FPC = 24          # full patches per core (8*24 = 192)
NPAIR = FPC // 2  # 12 pairs
NCHUNK = 6        # 768 / 128 contraction chunks

LAST_RESULTS = None    # BassKernelResults of the most recent run (for test.py)

_NC_CACHE = {}


def _build():
    nc = bacc.Bacc()
    aT = nc.declare_dram_parameter("aT", [128, FPC + 1, NCHUNK, B], bf16, isOutput=False)
    Wt = nc.declare_dram_parameter("Wt", [FPC, 128, NCHUNK, D], bf16, isOutput=False)
    Wh = nc.declare_dram_parameter("Wh", [128, NCHUNK, HD], bf16, isOutput=False)
    bhl = nc.declare_dram_parameter("bhl", [2, FPC, D], bf16, isOutput=False)
    bh = nc.declare_dram_parameter("bh", [2, HD], bf16, isOutput=False)
    outp = nc.declare_dram_parameter("outp", [NPAIR, 2 * B, D], bf16, isOutput=True)
    outh = nc.declare_dram_parameter("outh", [B, HD], bf16, isOutput=True)

    with tile.TileContext(nc) as tc:
        with (
            tc.tile_pool(name="const", bufs=1) as cpool,
            tc.tile_pool(name="a", bufs=6) as apool,
            tc.tile_pool(name="w", bufs=8) as wpool,
            tc.tile_pool(name="o", bufs=3) as opool,
            tc.tile_pool(name="ps", bufs=3, space="PSUM") as pspool,
            tc.tile_pool(name="psh", bufs=1, space="PSUM") as pshpool,
        ):
            ones = cpool.tile([2, B], bf16)
            nc.vector.memset(ones[:], 1.0)

            slices = [(0, 512), (512, 768)]

            def wtile(p):
                # Split each patch's W tile across BOTH HWDGE rings: a single
                # ring caps out near ~200 GB/s, well below the per-core DMA
                # engine pool, so W (the dominant traffic) must ride both.
                t = wpool.tile([128, NCHUNK, D], bf16, tag="wt")
                h = NCHUNK // 2
                nc.sync.dma_start(t[:, :h], Wt[p, :, :h])
                nc.scalar.dma_start(t[:, h:], Wt[p, :, h:])
                return t

            # Prefetch the half-patch job's inputs up front: they are tiny,
            # prime both queues while the first pair's descriptors generate,
            # and make the half-job compute at the end start with everything
            # already resident (short drain).
            wh = wpool.tile([128, NCHUNK, HD], bf16, tag="wh")
            hh = NCHUNK // 2
            nc.sync.dma_start(wh[:, :hh], Wh[:, :hh])
            nc.scalar.dma_start(wh[:, hh:], Wh[:, hh:])
            ah = apool.tile([128, 1, NCHUNK, B], bf16, tag="ah")
            tbh = apool.tile([2, HD], bf16, tag="tbh")
            nc.sync.dma_start(ah[:], aT[:, FPC:FPC + 1])
            nc.scalar.dma_start(tbh[:], bh[:, :])

            for j in range(NPAIR):
                p0, p1 = 2 * j, 2 * j + 1
                wt0 = wtile(p0)
                wt1 = wtile(p1)
                at = apool.tile([128, 2, NCHUNK, B], bf16, tag="at")
                tb = apool.tile([2, 2, D], bf16, tag="tb")
                # Alternate the small transfers between rings so both queues
                # stay byte-even throughout (outputs trail compute by a pair,
                # so pinning them to one ring leaves that ring lagging at the
                # tail).
                eng_a = nc.sync if j % 2 == 0 else nc.scalar
                eng_a.dma_start(at[:], aT[:, p0:p0 + 2])
                eng_a.dma_start(tb[:], bhl[:, p0:p0 + 2, :])
                a0 = at[:, 0]
                a1 = at[:, 1]

                pt = pspool.tile([2 * B, D], f32, tag="pt")
                for (o0, o1) in slices:
                    nc.tensor.matmul(
                        pt[:B, o0:o1], ones[:], tb[:, 0, o0:o1],
                        start=True, stop=False, tile_position=(0, 0),
                    )
                    nc.tensor.matmul(
                        pt[B:, o0:o1], ones[:], tb[:, 1, o0:o1],
                        start=True, stop=False, tile_position=(0, B),
                    )
                for c in range(NCHUNK):
                    for (o0, o1) in slices:
                        nc.tensor.matmul(
                            pt[:B, o0:o1], a0[:, c, :], wt0[:, c, o0:o1],
                            start=False, stop=(c == NCHUNK - 1),
                            tile_position=(0, 0),
                        )
                        nc.tensor.matmul(
                            pt[B:, o0:o1], a1[:, c, :], wt1[:, c, o0:o1],
                            start=False, stop=(c == NCHUNK - 1),
                            tile_position=(0, B),
                        )
                ob = opool.tile([2 * B, D], bf16, tag="ob")
                nc.vector.tensor_copy(ob[:], pt[:])
                eng_o = nc.scalar if j % 2 == 0 else nc.sync
                eng_o.dma_start(outp[j], ob[:])

            # Half-patch job: one patch, HD of its D outputs, runs last (its
            # inputs were prefetched before the pair loop) so the tail after
            # the final W byte is short.
            ph = pshpool.tile([B, HD], f32, tag="ph")
            nc.tensor.matmul(
                ph[:, :], ones[:], tbh[:, :],
                start=True, stop=False, tile_position=(0, 0),
            )
            for c in range(NCHUNK):
                nc.tensor.matmul(
                    ph[:, :], ah[:, 0, c, :], wh[:, c, :],
                    start=False, stop=(c == NCHUNK - 1),
                    tile_position=(0, 0),
                )
            oh = opool.tile([B, HD], bf16, tag="oh")
            nc.vector.tensor_copy(oh[:], ph[:])
            nc.scalar.dma_start(outh[:, :], oh[:])

    nc.finalize()
    return nc


def _patchify(x):
    # [B, C, H, W] -> [B, 196, 768] in MAE ordering (n c h p w q -> n h w p q c)
    Bn, C, H, Wd = x.shape
    h = H // 16
    xr = x.reshape(Bn, C, h, 16, h, 16)
    xr = np.transpose(xr, (0, 2, 4, 3, 5, 1))
    return xr.reshape(Bn, h * h, 16 * 16 * C)


def _bias_hilo(v):
    hi = v.astype(ml_dtypes.bfloat16)
    lo = (v - hi.astype(np.float32)).astype(ml_dtypes.bfloat16)
    return np.ascontiguousarray(np.stack([hi, lo], axis=0))


def kernel(x, W, b, _trace=False, _tmpdir=None):
    global LAST_RESULTS

    x = np.asarray(x, dtype=np.float32)
    W = np.asarray(W, dtype=np.float32)
    b = np.asarray(b, dtype=np.float32)

    patches = _patchify(x).astype(ml_dtypes.bfloat16)   # [64, 196, 768]
    Wb = W.astype(ml_dtypes.bfloat16)                   # [196, 768, 768]

    in_maps = []
    for k in range(N_CORES):
        idx = np.arange(k * FPC, (k + 1) * FPC)         # full patches
        hp = 8 * FPC + k // 2                           # shared half patch
        ho = (k % 2) * HD                               # its output offset

        psl = patches[:, list(idx) + [hp], :]           # [64, 25, 768]
        aT = np.ascontiguousarray(
            psl.transpose(2, 1, 0)                      # [768, 25, 64]
            .reshape(NCHUNK, 128, FPC + 1, B)
            .transpose(1, 2, 0, 3)                      # [128, 25, 6, 64]
        )
        wsl = Wb[idx]                                   # [24, 768, 768]
        Wt = np.ascontiguousarray(
            wsl.transpose(0, 2, 1)                      # [24, 768(i), 768(o)]
            .reshape(FPC, NCHUNK, 128, D)
            .transpose(0, 2, 1, 3)                      # [24, 128, 6, 768]
        )
        Wh = np.ascontiguousarray(
            Wb[hp, ho:ho + HD, :]                       # [384(o), 768(i)]
            .transpose(1, 0)                            # [768(i), 384(o)]
            .reshape(NCHUNK, 128, HD)
            .transpose(1, 0, 2)                         # [128, 6, 384]
        )
        in_maps.append({
            "aT": aT, "Wt": Wt, "Wh": Wh,
            "bhl": _bias_hilo(b[idx]), "bh": _bias_hilo(b[hp, ho:ho + HD]),
        })

    if "F" not in _NC_CACHE:
        _NC_CACHE["F"] = _build()
    nc = _NC_CACHE["F"]

    res = run_bass_kernel_spmd(
        nc, in_maps, list(range(N_CORES)), trace=_trace, tmpdir=_tmpdir
    )
    LAST_RESULTS = res

    out = np.empty((B, N_CORES * FPC + 4, D), dtype=np.float32)
    for k in range(N_CORES):
        op = res.results[k]["outp"].astype(np.float32)  # [12, 128, 768]
        out[:, k * FPC:(k + 1) * FPC, :] = (
            op.reshape(FPC, B, D).transpose(1, 0, 2)
        )
        hp = 8 * FPC + k // 2
        ho = (k % 2) * HD
        out[:, hp, ho:ho + HD] = res.results[k]["outh"].astype(np.float32)
    return np.ascontiguousarray(out[:, :NP, :])


# revision 15
# speedup vs baseline: 1.1832x; 1.1659x over previous
"""Trainium2 Bass kernel for nn_NonsharedPatchEmbed_86827058856432.

Computes, for a patchified [64, 3, 224, 224] fp32 image batch,

    out[b, p, o] = sum_i patches[b, p, i] * W[p, o, i] + bias[p, o]

with 196 independent Linear(768->768) layers (one per patch).

The problem is HBM-bound on W traffic (196*768*768 elements, each used
once per core under patch sharding), so W and the activations are cast to
bf16 on the host: this halves the dominant DMA bytes and quarters the
tensor-engine stream time (bf16 moves 1 col/cycle vs fp32's 4). PSUM
accumulation stays fp32; measured end-to-end relative error is ~4e-3,
well inside the 2e-2 gate.

Distribution: 196 = 8 * 24.5, so each core gets 24 full patches plus ONE
HALF of a shared patch (384 of its 768 outputs): patches 0-191 go 24 per
core, and patches 192-195 are split into 8 output-halves, one per core.
Every core therefore reads exactly 24.5/196 of W -- perfect balance, and
the per-core DMA-engine pool (16 engines x ~22.4 GB/s ~= 360 GB/s) is the
roofline. The half-patch job runs LAST so the post-last-W drain (compute +
PSUM copy + output write) is ~4x smaller than a full pair's.

Per-core kernel (column-tiled pairs):
  - 12 pairs of full patches, then the half-patch job.
  - For each pair, patch A owns PSUM partitions 0-63 (tile_position (0, 0)),
    patch B owns partitions 64-127 ((0, 64)). Each streams its own W^T as
    the moving operand; the batch activations (aT chunks, [128 x 64]) are
    the stationary operand.
  - The bias is applied with a K=2 bf16 matmul (ones x [bias_hi; bias_lo])
    that *starts* each PSUM accumulation group; the hi/lo split keeps the
    bias contribution bit-accurate and absorbs the PSUM WAR dependency.
  - A single HWDGE ring caps near ~200 GB/s, well below the 16-engine DMA
    pool, so W is split across BOTH rings every patch: chunks 0-2 ride the
    SP ring, chunks 3-5 the ACT ring. Activations+bias ride SP, outputs
    ride ACT, keeping the two rings byte-balanced (~17 MB each).
  - W dma_starts are issued before act/bias in each iteration so the
    big descriptors hit the queues first.

Layouts per core:
  aT  [128, 25, 6, 64]  bf16  aT[i, p, c, b] = patches[b, patch(p), 128c+i]
                              (p = 0..23 full patches, p = 24 half patch)
  Wt  [24, 128, 6, 768] bf16  Wt[p, i, c, o] = W[patch(p), o, 128c+i]
  Wh  [128, 6, 384]     bf16  half-patch W slice (output half `hp_off`)
  bhl [2, 24, 768]      bf16  bias split as hi + lo
  bh  [2, 384]          bf16  half-patch bias hi + lo
  outp [12, 128, 768]   bf16  pair j rows 0-63 -> patch 2j, 64-127 -> 2j+1
  outh [64, 384]        bf16  half-patch outputs
"""

import numpy as np
import ml_dtypes

import concourse.tile as tile
import concourse.mybir as mybir
from concourse import bacc
from concourse.bass_utils import run_bass_kernel_spmd

f32 = mybir.dt.float32
bf16 = mybir.dt.bfloat16

N_CORES = 8
B = 64            # batch
D = 768           # in/out feature dim
HD = 384          # half of D (half-patch job width)
NP = 196          # real patches
FPC = 24          # full patches per core (8*24 = 192)
NPAIR = FPC // 2  # 12 pairs
NCHUNK = 6        # 768 / 128 contraction chunks

LAST_RESULTS = None    # BassKernelResults of the most recent run (for test.py)

_NC_CACHE = {}


def _build():
    nc = bacc.Bacc()
    aT = nc.declare_dram_parameter("aT", [128, FPC + 1, NCHUNK, B], bf16, isOutput=False)
    Wt = nc.declare_dram_parameter("Wt", [FPC, 128, NCHUNK, D], bf16, isOutput=False)
    Wh = nc.declare_dram_parameter("Wh", [128, NCHUNK, HD], bf16, isOutput=False)
    bhl = nc.declare_dram_parameter("bhl", [2, FPC, D], bf16, isOutput=False)
    bh = nc.declare_dram_parameter("bh", [2, HD], bf16, isOutput=False)
    outp = nc.declare_dram_parameter("outp", [NPAIR, 2 * B, D], bf16, isOutput=True)
    outh = nc.declare_dram_parameter("outh", [B, HD], bf16, isOutput=True)

    with tile.TileContext(nc) as tc:
        with (
            tc.tile_pool(name="const", bufs=1) as cpool,
            tc.tile_pool(name="a", bufs=6) as apool,
            tc.tile_pool(name="w", bufs=8) as wpool,
            tc.tile_pool(name="o", bufs=3) as opool,
            tc.tile_pool(name="ps", bufs=3, space="PSUM") as pspool,
            tc.tile_pool(name="psh", bufs=1, space="PSUM") as pshpool,
        ):
            ones = cpool.tile([2, B], bf16)
            nc.vector.memset(ones[:], 1.0)

            slices = [(0, 512), (512, 768)]

            def wtile(p):
                # Split each patch's W tile across BOTH HWDGE rings: a single
                # ring caps out near ~200 GB/s, well below the per-core DMA
                # engine pool, so W (the dominant traffic) must ride both.
                t = wpool.tile([128, NCHUNK, D], bf16, tag="wt")
                h = NCHUNK // 2
                nc.sync.dma_start(t[:, :h], Wt[p, :, :h])
                nc.scalar.dma_start(t[:, h:], Wt[p, :, h:])
                return t

            # Prefetch the half-patch job's inputs up front: they are tiny,
            # prime both queues while the first pair's descriptors generate,
            # and make the half-job compute at the end start with everything
            # already resident (short drain).
            wh = wpool.tile([128, NCHUNK, HD], bf16, tag="wh")
            hh = NCHUNK // 2
            nc.sync.dma_start(wh[:, :hh], Wh[:, :hh])
            nc.scalar.dma_start(wh[:, hh:], Wh[:, hh:])
            ah = apool.tile([128, 1, NCHUNK, B], bf16, tag="ah")
            tbh = apool.tile([2, HD], bf16, tag="tbh")
            nc.sync.dma_start(ah[:], aT[:, FPC:FPC + 1])
            nc.scalar.dma_start(tbh[:], bh[:, :])

            for j in range(NPAIR):
                p0, p1 = 2 * j, 2 * j + 1
                wt0 = wtile(p0)
                wt1 = wtile(p1)
                at = apool.tile([128, 2, NCHUNK, B], bf16, tag="at")
                tb = apool.tile([2, 2, D], bf16, tag="tb")
                nc.sync.dma_start(at[:], aT[:, p0:p0 + 2])
                nc.sync.dma_start(tb[:], bhl[:, p0:p0 + 2, :])
                a0 = at[:, 0]
                a1 = at[:, 1]

                pt = pspool.tile([2 * B, D], f32, tag="pt")
                for (o0, o1) in slices:
                    nc.tensor.matmul(
                        pt[:B, o0:o1], ones[:], tb[:, 0, o0:o1],
                        start=True, stop=False, tile_position=(0, 0),
                    )
                    nc.tensor.matmul(
                        pt[B:, o0:o1], ones[:], tb[:, 1, o0:o1],
                        start=True, stop=False, tile_position=(0, B),
                    )
                for c in range(NCHUNK):
                    for (o0, o1) in slices:
                        nc.tensor.matmul(
                            pt[:B, o0:o1], a0[:, c, :], wt0[:, c, o0:o1],
                            start=False, stop=(c == NCHUNK - 1),
                            tile_position=(0, 0),
                        )
                        nc.tensor.matmul(
                            pt[B:, o0:o1], a1[:, c, :], wt1[:, c, o0:o1],
                            start=False, stop=(c == NCHUNK - 1),
                            tile_position=(0, B),
                        )
                ob = opool.tile([2 * B, D], bf16, tag="ob")
                nc.vector.tensor_copy(ob[:], pt[:])
                nc.scalar.dma_start(outp[j], ob[:])

            # Half-patch job: one patch, HD of its D outputs, runs last (its
            # inputs were prefetched before the pair loop) so the tail after
            # the final W byte is short.
            ph = pshpool.tile([B, HD], f32, tag="ph")
            nc.tensor.matmul(
                ph[:, :], ones[:], tbh[:, :],
                start=True, stop=False, tile_position=(0, 0),
            )
            for c in range(NCHUNK):
                nc.tensor.matmul(
                    ph[:, :], ah[:, 0, c, :], wh[:, c, :],
                    start=False, stop=(c == NCHUNK - 1),
                    tile_position=(0, 0),
                )
            oh = opool.tile([B, HD], bf16, tag="oh")
            nc.vector.tensor_copy(oh[:], ph[:])
            nc.scalar.dma_start(outh[:, :], oh[:])

    nc.finalize()
    return nc


def _patchify(x):
    # [B, C, H, W] -> [B, 196, 768] in MAE ordering (n c h p w q -> n h w p q c)
    Bn, C, H, Wd = x.shape
    h = H // 16
    xr = x.reshape(Bn, C, h, 16, h, 16)
    xr = np.transpose(xr, (0, 2, 4, 3, 5, 1))
    return xr.reshape(Bn, h * h, 16 * 16 * C)


def _bias_hilo(v):
    hi = v.astype(ml_dtypes.bfloat16)
    lo = (v - hi.astype(np.float32)).astype(ml_dtypes.bfloat16)
    return np.ascontiguousarray(np.stack([hi, lo], axis=0))


def kernel(x, W, b, _trace=False, _tmpdir=None):
    global LAST_RESULTS

    x = np.asarray(x, dtype=np.float32)
    W = np.asarray(W, dtype=np.float32)
    b = np.asarray(b, dtype=np.float32)

    patches = _patchify(x).astype(ml_dtypes.bfloat16)   # [64, 196, 768]
    Wb = W.astype(ml_dtypes.bfloat16)                   # [196, 768, 768]

    in_maps = []
    for k in range(N_CORES):
        idx = np.arange(k * FPC, (k + 1) * FPC)         # full patches
        hp = 8 * FPC + k // 2                           # shared half patch
        ho = (k % 2) * HD                               # its output offset

        psl = patches[:, list(idx) + [hp], :]           # [64, 25, 768]
        aT = np.ascontiguousarray(
            psl.transpose(2, 1, 0)                      # [768, 25, 64]
            .reshape(NCHUNK, 128, FPC + 1, B)
            .transpose(1, 2, 0, 3)                      # [128, 25, 6, 64]
        )
        wsl = Wb[idx]                                   # [24, 768, 768]
        Wt = np.ascontiguousarray(
            wsl.transpose(0, 2, 1)                      # [24, 768(i), 768(o)]
            .reshape(FPC, NCHUNK, 128, D)
            .transpose(0, 2, 1, 3)                      # [24, 128, 6, 768]
        )
        Wh = np.ascontiguousarray(
            Wb[hp, ho:ho + HD, :]                       # [384(o), 768(i)]
            .transpose(1, 0)                            # [768(i), 384(o)]
            .reshape(NCHUNK, 128, HD)
            .transpose(1, 0, 2)                         # [128, 6, 384]
        )
        in_maps.append({
            "aT": aT, "Wt": Wt, "Wh": Wh,
            "bhl": _bias_hilo(b[idx]), "bh": _bias_hilo(b[hp, ho:ho + HD]),
        })

    if "F" not in _NC_CACHE:
        _NC_CACHE["F"] = _build()
    nc = _NC_CACHE["F"]

    res = run_bass_kernel_spmd(
        nc, in_maps, list(range(N_CORES)), trace=_trace, tmpdir=_tmpdir
    )
    LAST_RESULTS = res

    out = np.empty((B, N_CORES * FPC + 4, D), dtype=np.float32)
    for k in range(N_CORES):
        op = res.results[k]["outp"].astype(np.float32)  # [12, 128, 768]
        out[:, k * FPC:(k + 1) * FPC, :] = (
            op.reshape(FPC, B, D).transpose(1, 0, 2)
        )
        hp = 8 * FPC + k // 2
        ho = (k % 2) * HD
        out[:, hp, ho:ho + HD] = res.results[k]["outh"].astype(np.float32)
    return np.ascontiguousarray(out[:, :NP, :])
